# revision 1
# baseline (speedup 1.0000x reference)
"""Trainium2 Bass kernel: ColumnParallelLinear + multi-adapter LoRA routing.

Computes out = x @ W^T + bias + B[aid[s]] @ (A[aid[s]] @ x[s]) for each token.

Distribution (tensor-parallel over d_out, 8 NeuronCores):
  - weight, bias, B_buffer sharded along d_out (512 features per core)
  - x, adapter_ids, A_buffer replicated
  - per-core output shard [512, S] gathered + transposed on host

Per-core kernel layout (all matmuls bf16, K=128 x M=128 x N=512 tiles):
  - host pre-transposes x to x^T [D_IN, S] so the contraction dim lands on
    SBUF partitions for both operands
  - A_buffer (reshaped [L*R=128, D_IN]) is concatenated below the weight
    shard, so the base matmul also produces xa = A_all @ x^T for free-form
    PE scheduling (one fused pass, PE never goes cold)
  - per-token adapter routing = precomputed {0,1} mask multiplied into the
    xa PSUM tile on the VectorE; the masked xa feeds a second matmul with
    B_cat^T that ACCUMULATES into the base PSUM tile (start=False)
  - bias is added during PSUM->SBUF eviction (per-partition tensor_scalar)
"""

import os
import sys

import numpy as np

try:
    import ml_dtypes
except ImportError:  # pragma: no cover
    sys.path.insert(0, "/opt/trn_rl_repo")
    import ml_dtypes

_P = 128  # SBUF partitions / matmul tile edge
_NT = 512  # token tile (matmul moving free dim, one PSUM bank of fp32)
_LR = 128  # L * R = 8 * 16 adapter-rank rows
_N_CORES = 8

_NC_CACHE = {}
LAST_RESULTS = None  # BassKernelResults of the most recent run (for test.py)


def _import_concourse():
    try:
        import concourse  # noqa: F401
    except ImportError:  # pragma: no cover
        for p in ("/opt/trn_rl_repo", "/root/.axon_site/_ro/trn_rl_repo"):
            if os.path.isdir(p) and p not in sys.path:
                sys.path.insert(0, p)


def build_nc(d_in: int, d_loc: int, s_tokens: int):
    """Build + finalize the per-core Bass kernel for the given shard shapes."""
    _import_concourse()
    import concourse.tile as tile
    from concourse import bacc, mybir

    P, NT, LR = _P, _NT, _LR
    n_kt = d_in // P
    n_mt = d_loc // P
    n_nt = s_tokens // NT
    assert d_in % P == 0 and d_loc % P == 0 and s_tokens % NT == 0

    nc = bacc.Bacc("TRN2", target_bir_lowering=False, debug=False)

    bf16 = mybir.dt.bfloat16
    f32 = mybir.dt.float32

    xT = nc.dram_tensor("xT", [d_in, s_tokens], bf16, kind="ExternalInput").ap()
    w_aug_t = nc.dram_tensor(
        "w_aug_t", [d_in, d_loc + LR], bf16, kind="ExternalInput"
    ).ap()
    b_cat_t = nc.dram_tensor("b_cat_t", [LR, d_loc], bf16, kind="ExternalInput").ap()
    maskT = nc.dram_tensor("maskT", [LR, s_tokens], bf16, kind="ExternalInput").ap()
    bias_pre = nc.dram_tensor("bias_pre", [P, n_mt], f32, kind="ExternalInput").ap()
    out_t = nc.dram_tensor("out_t", [d_loc, s_tokens], f32, kind="ExternalOutput").ap()

    # [d_in, n] with d_in = kt*128 + p  ->  [p, kt, n]
    xT_v = xT.rearrange("(kt p) s -> p kt s", p=P)
    w_v = w_aug_t.rearrange("(kt p) m -> p kt m", p=P)

    with tile.TileContext(nc) as tc:
        with (
            tc.tile_pool(name="const", bufs=1) as const_pool,
            tc.tile_pool(name="xp", bufs=1) as x_pool,
            tc.tile_pool(name="small", bufs=1) as small_pool,
            tc.tile_pool(name="outp", bufs=1) as out_pool,
            tc.tile_pool(name="psum", bufs=1, space="PSUM") as psum_pool,
        ):
            # Stationary operands, resident for the whole kernel.
            w_all = const_pool.tile([P, n_kt, d_loc + LR], bf16)
            for c in range(n_kt):
                nc.sync.dma_start(w_all[:, c, :], w_v[:, c, :])
            b_cat = const_pool.tile([P, d_loc], bf16)
            nc.sync.dma_start(b_cat[:], b_cat_t)
            bias_sb = const_pool.tile([P, n_mt], f32)
            nc.sync.dma_start(bias_sb[:], bias_pre)

            XCHUNK = 4  # k-tiles per x DMA -> 512KB chunks spread over queues
            for j in range(n_nt):
                tok = slice(j * NT, (j + 1) * NT)
                x_strip = x_pool.tile(
                    [P, n_kt, NT], bf16, tag="x_strip", bufs=3, name=f"x_strip{j}"
                )
                for c in range(0, n_kt, XCHUNK):
                    e = min(c + XCHUNK, n_kt)
                    nc.sync.dma_start(x_strip[:, c:e, :], xT_v[:, c:e, tok])
                mask_sb = small_pool.tile(
                    [P, NT], bf16, tag="mask", bufs=3, name=f"mask{j}"
                )
                nc.sync.dma_start(mask_sb[:], maskT[:, tok])

                # xa = A_all @ x^T for this token tile (all adapters at once)
                xa_ps = psum_pool.tile(
                    [P, NT], f32, tag="xa", bufs=2, name=f"xa_ps{j}"
                )
                for kt in range(n_kt):
                    nc.tensor.matmul(
                        xa_ps[:],
                        w_all[:, kt, d_loc : d_loc + LR],
                        x_strip[:, kt, :],
                        start=(kt == 0),
                        stop=(kt == n_kt - 1),
                    )
                xa_sb = small_pool.tile(
                    [P, NT], bf16, tag="xa_sb", bufs=3, name=f"xa_sb{j}"
                )
                nc.vector.tensor_mul(out=xa_sb[:], in0=xa_ps[:], in1=mask_sb[:])

                for m in range(n_mt):
                    ps = psum_pool.tile(
                        [P, NT], f32, tag="base", bufs=4, name=f"ps{j}_{m}"
                    )
                    for kt in range(n_kt):
                        nc.tensor.matmul(
                            ps[:],
                            w_all[:, kt, m * P : (m + 1) * P],
                            x_strip[:, kt, :],
                            start=(kt == 0),
                            stop=False,
                        )
                    # routed LoRA delta accumulates onto the base result
                    nc.tensor.matmul(
                        ps[:],
                        b_cat[:, m * P : (m + 1) * P],
                        xa_sb[:],
                        start=False,
                        stop=True,
                    )
                    o_sb = out_pool.tile(
                        [P, NT], f32, tag="o_sb", bufs=8, name=f"o_sb{j}_{m}"
                    )
                    nc.vector.tensor_scalar_add(
                        out=o_sb[:], in0=ps[:], scalar1=bias_sb[:, m : m + 1]
                    )
                    nc.sync.dma_start(out_t[m * P : (m + 1) * P, tok], o_sb[:])

    nc.finalize()
    return nc


def _get_nc(d_in, d_loc, s_tokens):
    key = (d_in, d_loc, s_tokens)
    if key not in _NC_CACHE:
        _NC_CACHE[key] = build_nc(*key)
    return _NC_CACHE[key]


def make_in_maps(x, adapter_ids, weight, bias, A_buffer, B_buffer, n_cores=_N_CORES):
    """Host-side shard + layout prep. Returns (in_maps, shapes)."""
    bf16 = ml_dtypes.bfloat16
    x = np.asarray(x, dtype=np.float32)
    adapter_ids = np.asarray(adapter_ids, dtype=np.int32)
    weight = np.asarray(weight, dtype=np.float32)
    bias = np.asarray(bias, dtype=np.float32)
    A_buffer = np.asarray(A_buffer, dtype=np.float32)
    B_buffer = np.asarray(B_buffer, dtype=np.float32)

    S, D_IN = x.shape
    D_OUT = weight.shape[0]
    L, R, _ = A_buffer.shape
    d_loc = D_OUT // n_cores
    LR = L * R
    assert LR == _LR

    xT = np.ascontiguousarray(x.astype(bf16).T)  # [D_IN, S]
    A_all = A_buffer.reshape(LR, D_IN)
    maskT = (np.arange(LR)[:, None] // R == adapter_ids[None, :]).astype(bf16)

    in_maps = []
    for i in range(n_cores):
        sl = slice(i * d_loc, (i + 1) * d_loc)
        w_aug = np.concatenate([weight[sl], A_all], axis=0)  # [d_loc+LR, D_IN]
        w_aug_t = np.ascontiguousarray(w_aug.astype(bf16).T)  # [D_IN, d_loc+LR]
        b_cat_t = np.ascontiguousarray(
            B_buffer[:, sl, :].transpose(0, 2, 1).reshape(LR, d_loc).astype(bf16)
        )
        bias_pre = np.ascontiguousarray(bias[sl].reshape(d_loc // _P, _P).T)
        in_maps.append(
            {
                "xT": xT,
                "w_aug_t": w_aug_t,
                "b_cat_t": b_cat_t,
                "maskT": maskT,
                "bias_pre": bias_pre,
            }
        )
    return in_maps, (S, D_IN, D_OUT, d_loc)


def kernel(x, adapter_ids, weight, bias, A_buffer, B_buffer):
    global LAST_RESULTS
    _import_concourse()
    from concourse.bass_utils import run_bass_kernel_spmd

    in_maps, (S, D_IN, D_OUT, d_loc) = make_in_maps(
        x, adapter_ids, weight, bias, A_buffer, B_buffer
    )
    nc = _get_nc(D_IN, d_loc, S)
    LAST_RESULTS = run_bass_kernel_spmd(
        nc, in_maps, core_ids=list(range(_N_CORES))
    )
    res = LAST_RESULTS.results
    out = np.empty((S, D_OUT), dtype=np.float32)
    for i in range(_N_CORES):
        out[:, i * d_loc : (i + 1) * d_loc] = res[i]["out_t"].T
    return out


# revision 2
# speedup vs baseline: 1.1467x; 1.1467x over previous
"""Trainium2 Bass kernel: ColumnParallelLinear + multi-adapter LoRA routing.

Computes out = x @ W^T + bias + B[aid[s]] @ (A[aid[s]] @ x[s]) for each token.

Distribution across 8 NeuronCores (one TRN2 chip):
  - base GEMM is tensor-parallel over d_out (sharding_hint): weight + bias
    sharded, each core emits out_base^T [512, S]
  - the LoRA delta is token-parallel: core i computes the delta for ITS
    1024-token slab across ALL d_out (A and B are tiny and replicated), so
    the rank-16 A-projection is computed once per token chip-wide instead
    of 8x replicated; no collectives needed — the host adds the two partial
    results while unsharding (out[s,o] = base[core o/512] + delta[core s/1024])

Per-core kernel (all matmuls bf16, K=128 tiles, N=512 moving):
  - host pre-transposes x so the contraction dim lands on SBUF partitions
  - per-token adapter routing = precomputed {0,1} mask multiplied into the
    xa PSUM tile on the VectorE before the B_cat matmul
  - bias is added during base PSUM->SBUF eviction (per-partition scalar add)
  - DMA emission is interleaved (w chunk k / x chunk k) so the first
    matmuls issue ~3us in instead of waiting for whole-tensor loads
"""

import os
import sys

import numpy as np

try:
    import ml_dtypes
except ImportError:  # pragma: no cover
    sys.path.insert(0, "/opt/trn_rl_repo")
    import ml_dtypes

_P = 128  # SBUF partitions / matmul tile edge
_NT = 512  # token tile (matmul moving free dim, one PSUM bank of fp32)
_LR = 128  # L * R = 8 * 16 adapter-rank rows
_N_CORES = 8

_NC_CACHE = {}
LAST_RESULTS = None  # BassKernelResults of the most recent run (for test.py)


def _import_concourse():
    try:
        import concourse  # noqa: F401
    except ImportError:  # pragma: no cover
        for p in ("/opt/trn_rl_repo", "/root/.axon_site/_ro/trn_rl_repo"):
            if os.path.isdir(p) and p not in sys.path:
                sys.path.insert(0, p)


def build_nc(d_in: int, d_loc: int, s_tokens: int, s_own: int, d_out: int):
    """Build + finalize the per-core Bass kernel.

    d_loc: output features of this core's base shard
    s_own: tokens in this core's LoRA-delta slab
    d_out: full output width (the delta covers all of it)
    """
    _import_concourse()
    import concourse.tile as tile
    from concourse import bacc, mybir

    P, NT, LR = _P, _NT, _LR
    n_kt = d_in // P
    n_mt = d_loc // P
    n_nt = s_tokens // NT
    n_ot = s_own // NT  # own-slab token tiles
    n_dt = d_out // P  # delta feature tiles
    assert all(v % P == 0 for v in (d_in, d_loc, d_out)) and s_tokens % NT == 0
    assert s_own % NT == 0

    nc = bacc.Bacc("TRN2", target_bir_lowering=False, debug=False)

    bf16 = mybir.dt.bfloat16
    f32 = mybir.dt.float32

    xT = nc.dram_tensor("xT", [d_in, s_tokens], bf16, kind="ExternalInput").ap()
    w_t = nc.dram_tensor("w_t", [d_in, d_loc], bf16, kind="ExternalInput").ap()
    a_t = nc.dram_tensor("a_t", [d_in, LR], bf16, kind="ExternalInput").ap()
    x_own_t = nc.dram_tensor("x_own_t", [d_in, s_own], bf16, kind="ExternalInput").ap()
    b_cat_t = nc.dram_tensor("b_cat_t", [LR, d_out], bf16, kind="ExternalInput").ap()
    mask_own = nc.dram_tensor("mask_own", [LR, s_own], bf16, kind="ExternalInput").ap()
    bias_pre = nc.dram_tensor("bias_pre", [P, n_mt], f32, kind="ExternalInput").ap()
    out_t = nc.dram_tensor("out_t", [d_loc, s_tokens], f32, kind="ExternalOutput").ap()
    delta_t = nc.dram_tensor("delta_t", [d_out, s_own], bf16, kind="ExternalOutput").ap()

    # [d_in, n] with d_in = kt*128 + p  ->  [p, kt, n]
    xT_v = xT.rearrange("(kt p) s -> p kt s", p=P)
    w_v = w_t.rearrange("(kt p) m -> p kt m", p=P)
    a_v = a_t.rearrange("(kt p) m -> p kt m", p=P)
    xo_v = x_own_t.rearrange("(kt p) s -> p kt s", p=P)

    XCHUNK = 4  # k-tiles per x/w DMA chunk

    with tile.TileContext(nc) as tc:
        with (
            tc.tile_pool(name="const", bufs=1) as const_pool,
            tc.tile_pool(name="xp", bufs=1) as x_pool,
            tc.tile_pool(name="small", bufs=1) as small_pool,
            tc.tile_pool(name="outp", bufs=1) as out_pool,
            tc.tile_pool(name="psum", bufs=1, space="PSUM") as psum_pool,
        ):
            w_all = const_pool.tile([P, n_kt, d_loc], bf16)
            b_cat = const_pool.tile([P, n_dt, P], bf16)
            bias_sb = const_pool.tile([P, n_mt], f32)
            a_all = const_pool.tile([P, n_kt, LR], bf16)
            xa_sb = const_pool.tile([P, s_own], bf16)
            mask_sb = const_pool.tile([P, s_own], bf16)

            def load_x_strip(j):
                x_strip = x_pool.tile(
                    [P, n_kt, NT], bf16, tag="x_strip", bufs=2, name=f"x_strip{j}"
                )
                tok = slice(j * NT, (j + 1) * NT)
                for c in range(0, n_kt, XCHUNK):
                    e = min(c + XCHUNK, n_kt)
                    nc.sync.dma_start(x_strip[:, c:e, :], xT_v[:, c:e, tok])
                return x_strip

            def base_ntile(j, x_strip):
                tok = slice(j * NT, (j + 1) * NT)
                for m in range(n_mt):
                    ps = psum_pool.tile(
                        [P, NT], f32, tag="base", bufs=4, name=f"ps{j}_{m}"
                    )
                    for kt in range(n_kt):
                        nc.tensor.matmul(
                            ps[:],
                            w_all[:, kt, m * P : (m + 1) * P],
                            x_strip[:, kt, :],
                            start=(kt == 0),
                            stop=(kt == n_kt - 1),
                        )
                    o_sb = out_pool.tile(
                        [P, NT], f32, tag="o_sb", bufs=6, name=f"o_sb{j}_{m}"
                    )
                    nc.vector.tensor_scalar_add(
                        out=o_sb[:], in0=ps[:], scalar1=bias_sb[:, m : m + 1]
                    )
                    nc.sync.dma_start(out_t[m * P : (m + 1) * P, tok], o_sb[:])

            # ---- startup: interleave w chunks with x-strip j=0 chunks so the
            # first base matmuls have their operands after ~1MB of DMA
            x_strip0 = x_pool.tile(
                [P, n_kt, NT], bf16, tag="x_strip", bufs=2, name="x_strip_first"
            )
            for c in range(0, n_kt, XCHUNK):
                e = min(c + XCHUNK, n_kt)
                nc.sync.dma_start(w_all[:, c:e, :], w_v[:, c:e, :])
                nc.sync.dma_start(x_strip0[:, c:e, :], xT_v[:, c:e, 0:NT])
            nc.sync.dma_start(bias_sb[:], bias_pre)

            base_ntile(0, x_strip0)

            # ---- prefetch LoRA operands during n-tile 0/1 compute
            for c in range(0, n_kt, XCHUNK):
                e = min(c + XCHUNK, n_kt)
                nc.sync.dma_start(a_all[:, c:e, :], a_v[:, c:e, :])
            nc.sync.dma_start(mask_sb[:], mask_own)
            for c in range(n_dt):
                nc.sync.dma_start(b_cat[:, c, :], b_cat_t[:, c * P : (c + 1) * P])

            base_ntile(1, load_x_strip(1))

            # ---- LoRA: xa = A_all @ x_own^T, masked per-token, then
            # delta^T = B_cat^T-matmul over ALL d_out for the own slab
            for n in range(n_ot):
                xo_strip = x_pool.tile(
                    [P, n_kt, NT], bf16, tag="x_own", bufs=1, name=f"xo_strip{n}"
                )
                for c in range(0, n_kt, XCHUNK):
                    e = min(c + XCHUNK, n_kt)
                    nc.sync.dma_start(
                        xo_strip[:, c:e, :], xo_v[:, c:e, n * NT : (n + 1) * NT]
                    )
                xa_ps = psum_pool.tile([P, NT], f32, tag="xa", bufs=2, name=f"xa_ps{n}")
                for kt in range(n_kt):
                    nc.tensor.matmul(
                        xa_ps[:],
                        a_all[:, kt, :],
                        xo_strip[:, kt, :],
                        start=(kt == 0),
                        stop=(kt == n_kt - 1),
                    )
                nc.vector.tensor_mul(
                    out=xa_sb[:, n * NT : (n + 1) * NT],
                    in0=xa_ps[:],
                    in1=mask_sb[:, n * NT : (n + 1) * NT],
                )
                for m in range(n_dt):
                    dl_ps = psum_pool.tile(
                        [P, NT], f32, tag="dl", bufs=2, name=f"dl_ps{n}_{m}"
                    )
                    nc.tensor.matmul(
                        dl_ps[:],
                        b_cat[:, m, :],
                        xa_sb[:, n * NT : (n + 1) * NT],
                        start=True,
                        stop=True,
                    )
                    d_sb = out_pool.tile(
                        [P, NT], bf16, tag="d_sb", bufs=4, name=f"d_sb{n}_{m}"
                    )
                    nc.scalar.copy(d_sb[:], dl_ps[:])
                    nc.sync.dma_start(
                        delta_t[m * P : (m + 1) * P, n * NT : (n + 1) * NT], d_sb[:]
                    )

            # ---- remaining base n-tiles
            for j in range(2, n_nt):
                base_ntile(j, load_x_strip(j))

    nc.finalize()
    return nc


def _get_nc(key):
    if key not in _NC_CACHE:
        _NC_CACHE[key] = build_nc(*key)
    return _NC_CACHE[key]


def make_in_maps(x, adapter_ids, weight, bias, A_buffer, B_buffer, n_cores=_N_CORES):
    """Host-side shard + layout prep. Returns (in_maps, shapes)."""
    bf16 = ml_dtypes.bfloat16
    x = np.asarray(x, dtype=np.float32)
    adapter_ids = np.asarray(adapter_ids, dtype=np.int32)
    weight = np.asarray(weight, dtype=np.float32)
    bias = np.asarray(bias, dtype=np.float32)
    A_buffer = np.asarray(A_buffer, dtype=np.float32)
    B_buffer = np.asarray(B_buffer, dtype=np.float32)

    S, D_IN = x.shape
    D_OUT = weight.shape[0]
    L, R, _ = A_buffer.shape
    d_loc = D_OUT // n_cores
    s_own = S // n_cores
    LR = L * R
    assert LR == _LR

    xT = np.ascontiguousarray(x.astype(bf16).T)  # [D_IN, S]
    a_t = np.ascontiguousarray(A_buffer.reshape(LR, D_IN).astype(bf16).T)
    b_cat_t = np.ascontiguousarray(
        B_buffer.transpose(0, 2, 1).reshape(LR, D_OUT).astype(bf16)
    )
    maskT = (np.arange(LR)[:, None] // R == adapter_ids[None, :]).astype(bf16)

    in_maps = []
    for i in range(n_cores):
        osl = slice(i * d_loc, (i + 1) * d_loc)
        tsl = slice(i * s_own, (i + 1) * s_own)
        w_t = np.ascontiguousarray(weight[osl].astype(bf16).T)  # [D_IN, d_loc]
        bias_pre = np.ascontiguousarray(bias[osl].reshape(d_loc // _P, _P).T)
        in_maps.append(
            {
                "xT": xT,
                "w_t": w_t,
                "a_t": a_t,
                "x_own_t": np.ascontiguousarray(xT[:, tsl]),
                "b_cat_t": b_cat_t,
                "mask_own": np.ascontiguousarray(maskT[:, tsl]),
                "bias_pre": bias_pre,
            }
        )
    return in_maps, (S, D_IN, D_OUT, d_loc, s_own)


def kernel(x, adapter_ids, weight, bias, A_buffer, B_buffer):
    global LAST_RESULTS
    _import_concourse()
    from concourse.bass_utils import run_bass_kernel_spmd

    in_maps, (S, D_IN, D_OUT, d_loc, s_own) = make_in_maps(
        x, adapter_ids, weight, bias, A_buffer, B_buffer
    )
    nc = _get_nc((D_IN, d_loc, S, s_own, D_OUT))
    LAST_RESULTS = run_bass_kernel_spmd(nc, in_maps, core_ids=list(range(_N_CORES)))
    res = LAST_RESULTS.results
    out = np.empty((S, D_OUT), dtype=np.float32)
    for i in range(_N_CORES):
        out[:, i * d_loc : (i + 1) * d_loc] = res[i]["out_t"].T
    for i in range(_N_CORES):
        out[i * s_own : (i + 1) * s_own, :] += res[i]["delta_t"].T.astype(np.float32)
    return out


# revision 5
# speedup vs baseline: 1.1761x; 1.0257x over previous
"""Trainium2 Bass kernel: ColumnParallelLinear + multi-adapter LoRA routing.

Computes out = x @ W^T + bias + B[aid[s]] @ (A[aid[s]] @ x[s]) for each token.

Distribution across 8 NeuronCores (one TRN2 chip):
  - base GEMM is tensor-parallel over d_out (sharding_hint): weight + bias
    sharded, each core emits out_base^T [512, S]
  - the LoRA delta is token-parallel: core i computes the delta for ITS
    1024-token slab across ALL d_out (A and B are tiny and replicated), so
    the rank-16 A-projection is computed once per token chip-wide instead
    of 8x replicated; no collectives needed — the host adds the two partial
    results while unsharding (out[s,o] = base[core o/512] + delta[core s/1024])
  - each core's token axis is ROTATED on the host so its own slab occupies
    the first two 512-token tiles; the xa matmuls then reuse the base
    x-strips already in SBUF (no extra x traffic, no prefetch stall), and
    the host un-rotates the base output during unsharding

Per-core kernel (all matmuls bf16, K=128 tiles, N=512 moving):
  - host pre-transposes x so the contraction dim lands on SBUF partitions
  - per-token adapter routing = precomputed {0,1} mask multiplied into the
    xa PSUM tile on the VectorE before the B_cat matmuls
  - bias is added during base PSUM->SBUF eviction (per-partition scalar add)
  - the 64 B_cat delta matmuls are drip-fed 2-per-base-m-tile so their
    PSUM-evict chain (ScalarE copy) never gates the PE
  - DMA emission is interleaved (w chunk k / x chunk k) with small leading
    chunks so the first matmul issues after ~256KB of DMA
"""

import os
import sys

import numpy as np

try:
    import ml_dtypes
except ImportError:  # pragma: no cover
    sys.path.insert(0, "/opt/trn_rl_repo")
    import ml_dtypes

_P = 128  # SBUF partitions / matmul tile edge
_NT = 512  # token tile (matmul moving free dim, one PSUM bank of fp32)
_LR = 128  # L * R = 8 * 16 adapter-rank rows
_N_CORES = 8

_NC_CACHE = {}
LAST_RESULTS = None  # BassKernelResults of the most recent run (for test.py)


def _import_concourse():
    try:
        import concourse  # noqa: F401
    except ImportError:  # pragma: no cover
        for p in ("/opt/trn_rl_repo", "/root/.axon_site/_ro/trn_rl_repo"):
            if os.path.isdir(p) and p not in sys.path:
                sys.path.insert(0, p)


def build_nc(d_in: int, d_loc: int, s_tokens: int, s_own: int, d_out: int):
    """Build + finalize the per-core Bass kernel.

    d_loc: output features of this core's base shard
    s_own: tokens in this core's LoRA-delta slab (the FIRST s_own tokens of
           the core's rotated token order)
    d_out: full output width (the delta covers all of it)
    """
    _import_concourse()
    import concourse.tile as tile
    from concourse import bacc, mybir

    P, NT, LR = _P, _NT, _LR
    n_kt = d_in // P
    n_mt = d_loc // P
    n_nt = s_tokens // NT
    n_ot = s_own // NT  # own-slab token tiles
    n_dt = d_out // P  # delta feature tiles
    assert all(v % P == 0 for v in (d_in, d_loc, d_out)) and s_tokens % NT == 0
    assert s_own % NT == 0 and n_ot <= n_nt

    nc = bacc.Bacc("TRN2", target_bir_lowering=False, debug=False)

    bf16 = mybir.dt.bfloat16
    f32 = mybir.dt.float32

    xT = nc.dram_tensor("xT", [d_in, s_tokens], bf16, kind="ExternalInput").ap()
    w_t = nc.dram_tensor("w_t", [d_in, d_loc], bf16, kind="ExternalInput").ap()
    a_t = nc.dram_tensor("a_t", [d_in, LR], bf16, kind="ExternalInput").ap()
    b_cat_t = nc.dram_tensor("b_cat_t", [LR, d_out], bf16, kind="ExternalInput").ap()
    mask_own = nc.dram_tensor("mask_own", [LR, s_own], bf16, kind="ExternalInput").ap()
    bias_pre = nc.dram_tensor("bias_pre", [P, n_mt], f32, kind="ExternalInput").ap()
    out_t = nc.dram_tensor("out_t", [d_loc, s_tokens], f32, kind="ExternalOutput").ap()
    delta_t = nc.dram_tensor("delta_t", [d_out, s_own], bf16, kind="ExternalOutput").ap()

    # [d_in, n] with d_in = kt*128 + p  ->  [p, kt, n]
    xT_v = xT.rearrange("(kt p) s -> p kt s", p=P)
    w_v = w_t.rearrange("(kt p) m -> p kt m", p=P)
    a_v = a_t.rearrange("(kt p) m -> p kt m", p=P)

    XCHUNK = 4  # k-tiles per x/w DMA chunk
    # finer chunks at the very start so the first matmul issues after ~256KB
    START_BOUNDS = [0, 1, 2, 3, 4]
    c = START_BOUNDS[-1]
    while c < n_kt:
        c = min(c + XCHUNK, n_kt)
        START_BOUNDS.append(c)
    START_BOUNDS = sorted(set(b for b in START_BOUNDS if b <= n_kt))

    with tile.TileContext(nc) as tc:
        with (
            tc.tile_pool(name="const", bufs=1) as const_pool,
            tc.tile_pool(name="xp", bufs=1) as x_pool,
            tc.tile_pool(name="outp", bufs=1) as out_pool,
            tc.tile_pool(name="psum", bufs=1, space="PSUM") as psum_pool,
        ):
            w_all = const_pool.tile([P, n_kt, d_loc], bf16)
            b_cat = const_pool.tile([P, n_dt, P], bf16)
            bias_sb = const_pool.tile([P, n_mt], f32)
            a_all = const_pool.tile([P, n_kt, LR], bf16)
            xa_sb = const_pool.tile([P, s_own], bf16)
            mask_sb = const_pool.tile([P, s_own], bf16)

            # Deferred LoRA-delta jobs, drip-fed between base m-tiles so the
            # PSUM-evict chain (ACT copy) never gates the PE.
            delta_jobs = []

            def emit_delta(k):
                for _ in range(k):
                    if not delta_jobs:
                        return
                    n, m = delta_jobs.pop(0)
                    dl_ps = psum_pool.tile(
                        [P, NT], f32, tag="dl", bufs=2, name=f"dl_ps{n}_{m}"
                    )
                    nc.tensor.matmul(
                        dl_ps[:],
                        b_cat[:, m, :],
                        xa_sb[:, n * NT : (n + 1) * NT],
                        start=True,
                        stop=True,
                    )
                    d_sb = out_pool.tile(
                        [P, NT], bf16, tag="d_sb", bufs=4, name=f"d_sb{n}_{m}"
                    )
                    nc.scalar.copy(d_sb[:], dl_ps[:])
                    nc.sync.dma_start(
                        delta_t[m * P : (m + 1) * P, n * NT : (n + 1) * NT], d_sb[:]
                    )

            def load_x_strip(j):
                x_strip = x_pool.tile(
                    [P, n_kt, NT], bf16, tag="x_strip", bufs=3, name=f"x_strip{j}"
                )
                tok = slice(j * NT, (j + 1) * NT)
                for c in range(0, n_kt, XCHUNK):
                    e = min(c + XCHUNK, n_kt)
                    nc.sync.dma_start(x_strip[:, c:e, :], xT_v[:, c:e, tok])
                return x_strip

            def base_ntile(j, x_strip):
                tok = slice(j * NT, (j + 1) * NT)
                for m in range(n_mt):
                    ps = psum_pool.tile(
                        [P, NT], f32, tag="base", bufs=4, name=f"ps{j}_{m}"
                    )
                    for kt in range(n_kt):
                        nc.tensor.matmul(
                            ps[:],
                            w_all[:, kt, m * P : (m + 1) * P],
                            x_strip[:, kt, :],
                            start=(kt == 0),
                            stop=(kt == n_kt - 1),
                        )
                    o_sb = out_pool.tile(
                        [P, NT], f32, tag="o_sb", bufs=6, name=f"o_sb{j}_{m}"
                    )
                    nc.vector.tensor_scalar_add(
                        out=o_sb[:], in0=ps[:], scalar1=bias_sb[:, m : m + 1]
                    )
                    nc.sync.dma_start(out_t[m * P : (m + 1) * P, tok], o_sb[:])
                    emit_delta(2)

            def xa_block(n, x_strip):
                # xa = A_all @ x^T for own-slab tile n, masked per-token;
                # queues that tile's 32 B_cat delta matmuls
                xa_ps = psum_pool.tile([P, NT], f32, tag="xa", bufs=2, name=f"xa_ps{n}")
                for kt in range(n_kt):
                    nc.tensor.matmul(
                        xa_ps[:],
                        a_all[:, kt, :],
                        x_strip[:, kt, :],
                        start=(kt == 0),
                        stop=(kt == n_kt - 1),
                    )
                nc.vector.tensor_mul(
                    out=xa_sb[:, n * NT : (n + 1) * NT],
                    in0=xa_ps[:],
                    in1=mask_sb[:, n * NT : (n + 1) * NT],
                )
                delta_jobs.extend((n, m) for m in range(n_dt))

            # ---- startup: interleave w chunks with x-strip j=0 chunks so the
            # first base matmuls have their operands after ~256KB of DMA
            x_strip0 = x_pool.tile(
                [P, n_kt, NT], bf16, tag="x_strip", bufs=3, name="x_strip_first"
            )
            for c, e in zip(START_BOUNDS, START_BOUNDS[1:]):
                nc.sync.dma_start(w_all[:, c:e, :], w_v[:, c:e, :])
                nc.sync.dma_start(x_strip0[:, c:e, :], xT_v[:, c:e, 0:NT])
            nc.sync.dma_start(bias_sb[:], bias_pre)
            # LoRA constants (a few MB; needed from ~35us in)
            for c in range(0, n_kt, XCHUNK):
                e = min(c + XCHUNK, n_kt)
                nc.sync.dma_start(a_all[:, c:e, :], a_v[:, c:e, :])
            nc.sync.dma_start(mask_sb[:], mask_own)
            for c in range(n_dt):
                nc.sync.dma_start(b_cat[:, c, :], b_cat_t[:, c * P : (c + 1) * P])

            strips = {0: x_strip0}
            for j in range(n_nt):
                x_strip = strips.pop(j) if j in strips else load_x_strip(j)
                base_ntile(j, x_strip)
                if j < n_ot:
                    xa_block(j, x_strip)
            while delta_jobs:
                emit_delta(len(delta_jobs))

    nc.finalize()
    return nc


def _get_nc(key):
    if key not in _NC_CACHE:
        _NC_CACHE[key] = build_nc(*key)
    return _NC_CACHE[key]


def make_in_maps(x, adapter_ids, weight, bias, A_buffer, B_buffer, n_cores=_N_CORES):
    """Host-side shard + layout prep. Returns (in_maps, shapes)."""
    bf16 = ml_dtypes.bfloat16
    x = np.asarray(x, dtype=np.float32)
    adapter_ids = np.asarray(adapter_ids, dtype=np.int32)
    weight = np.asarray(weight, dtype=np.float32)
    bias = np.asarray(bias, dtype=np.float32)
    A_buffer = np.asarray(A_buffer, dtype=np.float32)
    B_buffer = np.asarray(B_buffer, dtype=np.float32)

    S, D_IN = x.shape
    D_OUT = weight.shape[0]
    L, R, _ = A_buffer.shape
    d_loc = D_OUT // n_cores
    s_own = S // n_cores
    LR = L * R
    assert LR == _LR

    xT = np.ascontiguousarray(x.astype(bf16).T)  # [D_IN, S]
    a_t = np.ascontiguousarray(A_buffer.reshape(LR, D_IN).astype(bf16).T)
    b_cat_t = np.ascontiguousarray(
        B_buffer.transpose(0, 2, 1).reshape(LR, D_OUT).astype(bf16)
    )
    maskT = (np.arange(LR)[:, None] // R == adapter_ids[None, :]).astype(bf16)

    in_maps = []
    for i in range(n_cores):
        osl = slice(i * d_loc, (i + 1) * d_loc)
        w_t = np.ascontiguousarray(weight[osl].astype(bf16).T)  # [D_IN, d_loc]
        bias_pre = np.ascontiguousarray(bias[osl].reshape(d_loc // _P, _P).T)
        # rotate the token axis so core i's own slab comes first
        xT_rot = np.roll(xT, -i * s_own, axis=1) if i else xT
        in_maps.append(
            {
                "xT": np.ascontiguousarray(xT_rot),
                "w_t": w_t,
                "a_t": a_t,
                "b_cat_t": b_cat_t,
                "mask_own": np.ascontiguousarray(
                    maskT[:, i * s_own : (i + 1) * s_own]
                ),
                "bias_pre": bias_pre,
            }
        )
    return in_maps, (S, D_IN, D_OUT, d_loc, s_own)


def kernel(x, adapter_ids, weight, bias, A_buffer, B_buffer):
    global LAST_RESULTS
    _import_concourse()
    from concourse.bass_utils import run_bass_kernel_spmd

    in_maps, (S, D_IN, D_OUT, d_loc, s_own) = make_in_maps(
        x, adapter_ids, weight, bias, A_buffer, B_buffer
    )
    nc = _get_nc((D_IN, d_loc, S, s_own, D_OUT))
    LAST_RESULTS = run_bass_kernel_spmd(nc, in_maps, core_ids=list(range(_N_CORES)))
    res = LAST_RESULTS.results
    out = np.empty((S, D_OUT), dtype=np.float32)
    for i in range(_N_CORES):
        # un-rotate this core's token axis while scattering its base shard
        base = res[i]["out_t"]
        if i:
            base = np.roll(base, i * s_own, axis=1)
        out[:, i * d_loc : (i + 1) * d_loc] = base.T
    for i in range(_N_CORES):
        out[i * s_own : (i + 1) * s_own, :] += res[i]["delta_t"].T.astype(np.float32)
    return out


# revision 6
# speedup vs baseline: 1.2201x; 1.0374x over previous
"""Trainium2 Bass kernel: ColumnParallelLinear + multi-adapter LoRA routing.

Computes out = x @ W^T + bias + B[aid[s]] @ (A[aid[s]] @ x[s]) for each token.

Distribution across 8 NeuronCores (one TRN2 chip):
  - base GEMM is tensor-parallel over d_out (sharding_hint): weight + bias
    sharded, each core emits out_base^T [512, S]
  - the LoRA delta is token-parallel: core i computes the delta for ITS
    1024-token slab across ALL d_out (A and B are tiny and replicated), so
    the rank-16 A-projection is computed once per token chip-wide instead
    of 8x replicated; no collectives needed — the host adds the two partial
    results while unsharding (out[s,o] = base[core o/512] + delta[core s/1024])
  - each core's token axis is ROTATED on the host so its own slab occupies
    the first two 512-token tiles; the xa matmuls then reuse the base
    x-strips already in SBUF (no extra x traffic, no prefetch stall), and
    the host un-rotates the base output during unsharding

Per-core kernel (all matmuls bf16, K=128 tiles, N=512 moving):
  - host pre-transposes x so the contraction dim lands on SBUF partitions
  - per-token adapter routing = precomputed {0,1} mask multiplied into the
    xa PSUM tile on the VectorE before the B_cat matmuls
  - bias is added during base PSUM->SBUF eviction (per-partition scalar add)
  - the 64 B_cat delta matmuls are drip-fed 2-per-base-m-tile so their
    PSUM-evict chain (ScalarE copy) never gates the PE
  - DMA emission is interleaved (w chunk k / x chunk k) with small leading
    chunks so the first matmul issues after ~256KB of DMA
"""

import os
import sys

import numpy as np

try:
    import ml_dtypes
except ImportError:  # pragma: no cover
    sys.path.insert(0, "/opt/trn_rl_repo")
    import ml_dtypes

_P = 128  # SBUF partitions / matmul tile edge
_NT = 512  # token tile (matmul moving free dim, one PSUM bank of fp32)
_LR = 128  # L * R = 8 * 16 adapter-rank rows
_N_CORES = 8

_NC_CACHE = {}
LAST_RESULTS = None  # BassKernelResults of the most recent run (for test.py)


def _import_concourse():
    try:
        import concourse  # noqa: F401
    except ImportError:  # pragma: no cover
        for p in ("/opt/trn_rl_repo", "/root/.axon_site/_ro/trn_rl_repo"):
            if os.path.isdir(p) and p not in sys.path:
                sys.path.insert(0, p)


def build_nc(d_in: int, d_loc: int, s_tokens: int, s_own: int, d_out: int):
    """Build + finalize the per-core Bass kernel.

    d_loc: output features of this core's base shard
    s_own: tokens in this core's LoRA-delta slab (the FIRST s_own tokens of
           the core's rotated token order)
    d_out: full output width (the delta covers all of it)
    """
    _import_concourse()
    import concourse.tile as tile
    from concourse import bacc, mybir

    P, NT, LR = _P, _NT, _LR
    n_kt = d_in // P
    n_mt = d_loc // P
    n_nt = s_tokens // NT
    n_ot = s_own // NT  # own-slab token tiles
    n_dt = d_out // P  # delta feature tiles
    assert all(v % P == 0 for v in (d_in, d_loc, d_out)) and s_tokens % NT == 0
    assert s_own % NT == 0 and n_ot <= n_nt

    nc = bacc.Bacc("TRN2", target_bir_lowering=False, debug=False)

    bf16 = mybir.dt.bfloat16
    f32 = mybir.dt.float32

    xT = nc.dram_tensor("xT", [d_in, s_tokens], bf16, kind="ExternalInput").ap()
    w_t = nc.dram_tensor("w_t", [d_in, d_loc], bf16, kind="ExternalInput").ap()
    a_t = nc.dram_tensor("a_t", [d_in, LR], bf16, kind="ExternalInput").ap()
    b_cat_t = nc.dram_tensor("b_cat_t", [LR, d_out], bf16, kind="ExternalInput").ap()
    mask_own = nc.dram_tensor("mask_own", [LR, s_own], bf16, kind="ExternalInput").ap()
    bias_pre = nc.dram_tensor("bias_pre", [P, n_mt], f32, kind="ExternalInput").ap()
    out_t = nc.dram_tensor("out_t", [d_loc, s_tokens], f32, kind="ExternalOutput").ap()
    delta_t = nc.dram_tensor("delta_t", [d_out, s_own], bf16, kind="ExternalOutput").ap()

    # [d_in, n] with d_in = kt*128 + p  ->  [p, kt, n]
    xT_v = xT.rearrange("(kt p) s -> p kt s", p=P)
    w_v = w_t.rearrange("(kt p) m -> p kt m", p=P)
    a_v = a_t.rearrange("(kt p) m -> p kt m", p=P)

    XCHUNK = 4  # k-tiles per x/w DMA chunk
    # finer chunks at the very start so the first matmul issues after ~256KB
    START_BOUNDS = [0, 1, 2, 3, 4]
    c = START_BOUNDS[-1]
    while c < n_kt:
        c = min(c + XCHUNK, n_kt)
        START_BOUNDS.append(c)
    START_BOUNDS = sorted(set(b for b in START_BOUNDS if b <= n_kt))

    with tile.TileContext(nc) as tc:
        with (
            tc.tile_pool(name="const", bufs=1) as const_pool,
            tc.tile_pool(name="xp", bufs=1) as x_pool,
            tc.tile_pool(name="outp", bufs=1) as out_pool,
            tc.tile_pool(name="psum", bufs=1, space="PSUM") as psum_pool,
        ):
            w_all = const_pool.tile([P, n_kt, d_loc], bf16)
            b_cat = const_pool.tile([P, n_dt, P], bf16)
            bias_sb = const_pool.tile([P, n_mt], f32)
            a_all = const_pool.tile([P, n_kt, LR], bf16)
            xa_sb = const_pool.tile([P, s_own], bf16)
            mask_sb = const_pool.tile([P, s_own], bf16)

            # Deferred LoRA-delta jobs, drip-fed between base m-tiles so the
            # PSUM-evict chain (ACT copy) never gates the PE.
            delta_jobs = []

            def emit_delta(k):
                for _ in range(k):
                    if not delta_jobs:
                        return
                    n, m = delta_jobs.pop(0)
                    dl_ps = psum_pool.tile(
                        [P, NT], f32, tag="dl", bufs=2, name=f"dl_ps{n}_{m}"
                    )
                    nc.tensor.matmul(
                        dl_ps[:],
                        b_cat[:, m, :],
                        xa_sb[:, n * NT : (n + 1) * NT],
                        start=True,
                        stop=True,
                    )
                    d_sb = out_pool.tile(
                        [P, NT], bf16, tag="d_sb", bufs=4, name=f"d_sb{n}_{m}"
                    )
                    nc.scalar.copy(d_sb[:], dl_ps[:])
                    nc.sync.dma_start(
                        delta_t[m * P : (m + 1) * P, n * NT : (n + 1) * NT], d_sb[:]
                    )

            def load_x_strip(j):
                x_strip = x_pool.tile(
                    [P, n_kt, NT], bf16, tag="x_strip", bufs=3, name=f"x_strip{j}"
                )
                tok = slice(j * NT, (j + 1) * NT)
                for c in range(0, n_kt, XCHUNK):
                    e = min(c + XCHUNK, n_kt)
                    nc.sync.dma_start(x_strip[:, c:e, :], xT_v[:, c:e, tok])
                return x_strip

            def base_ntile(j, x_strip):
                tok = slice(j * NT, (j + 1) * NT)
                for m in range(n_mt):
                    ps = psum_pool.tile(
                        [P, NT], f32, tag="base", bufs=4, name=f"ps{j}_{m}"
                    )
                    for kt in range(n_kt):
                        nc.tensor.matmul(
                            ps[:],
                            w_all[:, kt, m * P : (m + 1) * P],
                            x_strip[:, kt, :],
                            start=(kt == 0),
                            stop=(kt == n_kt - 1),
                        )
                    o_sb = out_pool.tile(
                        [P, NT], f32, tag="o_sb", bufs=6, name=f"o_sb{j}_{m}"
                    )
                    nc.vector.tensor_scalar_add(
                        out=o_sb[:], in0=ps[:], scalar1=bias_sb[:, m : m + 1]
                    )
                    nc.sync.dma_start(out_t[m * P : (m + 1) * P, tok], o_sb[:])
                    emit_delta(2)

            def xa_block(n, x_strip):
                # xa = A_all @ x^T for own-slab tile n, masked per-token;
                # queues that tile's 32 B_cat delta matmuls
                xa_ps = psum_pool.tile([P, NT], f32, tag="xa", bufs=2, name=f"xa_ps{n}")
                for kt in range(n_kt):
                    nc.tensor.matmul(
                        xa_ps[:],
                        a_all[:, kt, :],
                        x_strip[:, kt, :],
                        start=(kt == 0),
                        stop=(kt == n_kt - 1),
                    )
                nc.vector.tensor_mul(
                    out=xa_sb[:, n * NT : (n + 1) * NT],
                    in0=xa_ps[:],
                    in1=mask_sb[:, n * NT : (n + 1) * NT],
                )
                delta_jobs.extend((n, m) for m in range(n_dt))

            # ---- startup: interleave w chunks with x-strip j=0 chunks so the
            # first base matmuls have their operands after ~128KB of DMA;
            # the leading single-k-tile chunks are split in half across two
            # DMA engines to halve their arrival latency
            x_strip0 = x_pool.tile(
                [P, n_kt, NT], bf16, tag="x_strip", bufs=3, name="x_strip_first"
            )
            for c, e in zip(START_BOUNDS, START_BOUNDS[1:]):
                if e - c == 1:
                    h = d_loc // 2
                    nc.sync.dma_start(w_all[:, c, :h], w_v[:, c, :h])
                    nc.sync.dma_start(w_all[:, c, h:], w_v[:, c, h:])
                    nc.sync.dma_start(x_strip0[:, c, : NT // 2], xT_v[:, c, : NT // 2])
                    nc.sync.dma_start(
                        x_strip0[:, c, NT // 2 :], xT_v[:, c, NT // 2 : NT]
                    )
                else:
                    nc.sync.dma_start(w_all[:, c:e, :], w_v[:, c:e, :])
                    nc.sync.dma_start(x_strip0[:, c:e, :], xT_v[:, c:e, 0:NT])
            nc.sync.dma_start(bias_sb[:], bias_pre)
            # warm the strip prefetch pipeline before any compute is emitted
            # (fresh pool slots -> these issue immediately on the Sync engine)
            strips = {0: x_strip0}
            for j in (1, 2):
                if j < n_nt:
                    strips[j] = load_x_strip(j)
            # LoRA constants (a few MB; needed from ~40us in)
            for c in range(0, n_kt, XCHUNK):
                e = min(c + XCHUNK, n_kt)
                nc.sync.dma_start(a_all[:, c:e, :], a_v[:, c:e, :])
            nc.sync.dma_start(mask_sb[:], mask_own)
            for c in range(n_dt):
                nc.sync.dma_start(b_cat[:, c, :], b_cat_t[:, c * P : (c + 1) * P])

            for j in range(n_nt):
                x_strip = strips.pop(j) if j in strips else load_x_strip(j)
                base_ntile(j, x_strip)
                if j < n_ot:
                    xa_block(j, x_strip)
            while delta_jobs:
                emit_delta(len(delta_jobs))

    nc.finalize()
    return nc


def _get_nc(key):
    if key not in _NC_CACHE:
        _NC_CACHE[key] = build_nc(*key)
    return _NC_CACHE[key]


def make_in_maps(x, adapter_ids, weight, bias, A_buffer, B_buffer, n_cores=_N_CORES):
    """Host-side shard + layout prep. Returns (in_maps, shapes)."""
    bf16 = ml_dtypes.bfloat16
    x = np.asarray(x, dtype=np.float32)
    adapter_ids = np.asarray(adapter_ids, dtype=np.int32)
    weight = np.asarray(weight, dtype=np.float32)
    bias = np.asarray(bias, dtype=np.float32)
    A_buffer = np.asarray(A_buffer, dtype=np.float32)
    B_buffer = np.asarray(B_buffer, dtype=np.float32)

    S, D_IN = x.shape
    D_OUT = weight.shape[0]
    L, R, _ = A_buffer.shape
    d_loc = D_OUT // n_cores
    s_own = S // n_cores
    LR = L * R
    assert LR == _LR

    xT = np.ascontiguousarray(x.astype(bf16).T)  # [D_IN, S]
    a_t = np.ascontiguousarray(A_buffer.reshape(LR, D_IN).astype(bf16).T)
    b_cat_t = np.ascontiguousarray(
        B_buffer.transpose(0, 2, 1).reshape(LR, D_OUT).astype(bf16)
    )
    maskT = (np.arange(LR)[:, None] // R == adapter_ids[None, :]).astype(bf16)

    in_maps = []
    for i in range(n_cores):
        osl = slice(i * d_loc, (i + 1) * d_loc)
        w_t = np.ascontiguousarray(weight[osl].astype(bf16).T)  # [D_IN, d_loc]
        bias_pre = np.ascontiguousarray(bias[osl].reshape(d_loc // _P, _P).T)
        # rotate the token axis so core i's own slab comes first
        xT_rot = np.roll(xT, -i * s_own, axis=1) if i else xT
        in_maps.append(
            {
                "xT": np.ascontiguousarray(xT_rot),
                "w_t": w_t,
                "a_t": a_t,
                "b_cat_t": b_cat_t,
                "mask_own": np.ascontiguousarray(
                    maskT[:, i * s_own : (i + 1) * s_own]
                ),
                "bias_pre": bias_pre,
            }
        )
    return in_maps, (S, D_IN, D_OUT, d_loc, s_own)


def kernel(x, adapter_ids, weight, bias, A_buffer, B_buffer):
    global LAST_RESULTS
    _import_concourse()
    from concourse.bass_utils import run_bass_kernel_spmd

    in_maps, (S, D_IN, D_OUT, d_loc, s_own) = make_in_maps(
        x, adapter_ids, weight, bias, A_buffer, B_buffer
    )
    nc = _get_nc((D_IN, d_loc, S, s_own, D_OUT))
    LAST_RESULTS = run_bass_kernel_spmd(nc, in_maps, core_ids=list(range(_N_CORES)))
    res = LAST_RESULTS.results
    out = np.empty((S, D_OUT), dtype=np.float32)
    for i in range(_N_CORES):
        # un-rotate this core's token axis while scattering its base shard
        base = res[i]["out_t"]
        if i:
            base = np.roll(base, i * s_own, axis=1)
        out[:, i * d_loc : (i + 1) * d_loc] = base.T
    for i in range(_N_CORES):
        out[i * s_own : (i + 1) * s_own, :] += res[i]["delta_t"].T.astype(np.float32)
    return out
